# revision 9
# baseline (speedup 1.0000x reference)
"""Bass/Tile kernel for BertUnpadSelfAttention on 8 TRN2 cores (v2).

Problem shapes: B=4, S=1024, L=512 valid tokens/seq, H=12, D=64, DIM=768.
Sharding: core c handles batch b=c//2, heads h0=6*(c%2) .. h0+5.

Key structure (vs v1 baseline):
  - bias is factored on the host: probs = exp(S) * exp(bias).  exp(bias) for
    the 512 valid keys ships as bf16 "EB"; the padded-key contribution to the
    softmax denominator is a pure function of bias and is reduced on the host
    (den_pad), so the padded half of bias never touches the device.
  - projection is chunk-streamed: the 6 contraction chunks of xw are DMA'd
    separately and every open PSUM accumulation group advances as each chunk
    lands, so PE overlaps the input stream.
  - QK^T uses K=64 row-tiled matmul pairs (even head in partitions 0-63,
    odd head in 64-127 -> concurrent T0/T8 tiles).
  - PV uses v augmented with a ones column: psc rows 0-63 = ctx, row 64 =
    sum of valid probs.  The final division (and adding den_pad) happens on
    the host, so the whole extract/recip/broadcast chain is gone.

Per-core device program:
  proj:  qk[f,t]   = sum_i w[i,f] x[i,t]      (6 groups, chunk-streamed)
         v[t,f]    = sum_i x[i,t] w[i,f]      (4 token blocks)
  per head j:  S[k,q] = kT_j.T-contract qT_j          (4 chunks of 128 keys)
               E      = exp(S)      (ACT, PSUM->SBUF bf16)
               probs  = E * EB_j    (DVE, 2x bf16)
               psc    = [v_j | 1].T @ probs            (65 x 512, PSUM f32)
               out[j] = psc                            (DMA f32)
"""
import sys

sys.path.insert(0, "/opt/trn_rl_repo")

import numpy as np

import concourse.bacc as bacc
import concourse.mybir as mybir
from concourse.tile import TileContext

F32 = mybir.dt.float32
BF16 = mybir.dt.bfloat16
import os as _os
import ml_dtypes as _mld

MM_DT = BF16
MM_NP = _mld.bfloat16
P = 128
B, S, L = 4, 1024, 512
H, D = 12, 64
DIM = H * D          # 768
HPC = 6              # heads per core
T = 512              # tokens per core
NKC = 4              # valid-key chunks of 128
NPAIR = HPC // 2
KC_IN = DIM // P     # 6 contraction chunks
XWC = T + 2 * HPC * D + HPC * D   # 512 + 768 + 384 = 1664
SCALE = 1.0 / 8.0
WARM_MMS = int(_os.environ.get("ATTN_WARM", "10"))
EXP = mybir.ActivationFunctionType.Exp


def build_kernel(skip_qkv_bias=True):
    nc = bacc.Bacc("TRN2", target_bir_lowering=False, debug=False, num_devices=8)

    xw = nc.dram_tensor("xw", [DIM, XWC], MM_DT, kind="ExternalInput")
    eb = nc.dram_tensor("eb", [HPC, P, NKC * T], MM_DT, kind="ExternalInput")
    bqk = nc.dram_tensor("bqk", [1, 2 * HPC * D], MM_DT, kind="ExternalInput")
    bv = nc.dram_tensor("bv", [1, HPC * D], MM_DT, kind="ExternalInput")
    out = nc.dram_tensor("out", [HPC, D + 1, T], F32, kind="ExternalOutput")

    with TileContext(nc) as tc:
        with (
            tc.tile_pool(name="const", bufs=1) as cpool,
            tc.tile_pool(name="qkv", bufs=1) as qkvpool,
            tc.tile_pool(name="eprob", bufs=2) as epool,
            tc.tile_pool(name="ps", bufs=3, space="PSUM") as pspool,
            tc.tile_pool(name="psc", bufs=1, space="PSUM") as pscpool,
            tc.tile_pool(name="psv", bufs=1, space="PSUM") as psvpool,
        ):
            # ---- HAM warm-up as early as possible ----
            warm_w = cpool.tile([P, D], MM_DT, tag="warm_w")
            nc.vector.memset(warm_w[:], 0.0)
            warm_x = cpool.tile([P, T], MM_DT, tag="warm_x")
            nc.vector.memset(warm_x[:], 0.0)
            for wi in range(WARM_MMS):
                pw = pscpool.tile([D + 1, T], F32, tag="psc", name=f"pw{wi}")
                nc.tensor.matmul(pw[0:D, 0:P], warm_w[:], warm_x[:, 0:P],
                                 start=True, stop=True)
            # trigger the exp table-set load while input DMAs stream
            aw = cpool.tile([P, 16], F32, tag="aw")
            nc.vector.memset(aw[:], 0.0)
            awo = cpool.tile([P, 16], MM_DT, tag="awo")
            nc.scalar.activation(awo[:], aw[:], EXP)
            onesix = cpool.tile([P, HPC], MM_DT, tag="onesix")
            nc.vector.memset(onesix[:], 1.0)
            if not skip_qkv_bias:
                ones1 = cpool.tile([1, T], MM_DT, tag="ones1")
                nc.vector.memset(ones1[:], 1.0)
                bqk_sb = cpool.tile([1, 2 * HPC * D], MM_DT, tag="bqk")
                nc.sync.dma_start(out=bqk_sb[:], in_=bqk[:])
                bv_sb = cpool.tile([1, HPC * D], MM_DT, tag="bv")
                nc.sync.dma_start(out=bv_sb[:], in_=bv[:])

            # ---- input DMAs (sync queue, in priority order) ----
            xw_sb = []
            for kc in range(KC_IN):
                xt = cpool.tile([P, XWC], MM_DT, tag=f"xw{kc}", name=f"xt{kc}")
                nc.sync.dma_start(out=xt[:], in_=xw[kc * P:(kc + 1) * P, :])
                xw_sb.append(xt)
            eb_sb = []
            for j in range(HPC):
                et = cpool.tile([P, NKC * T], MM_DT, tag=f"eb{j}", name=f"et{j}")
                nc.sync.dma_start(out=et[:], in_=eb[j])
                eb_sb.append(et)

            # ---- projection: chunk-streamed qk groups ----
            # pair p's psum slot: [:, 0:T] = q heads (2p,2p+1), [:, T:2T] = k
            pair_ps = []
            for p_ in range(NPAIR):
                pp = pspool.tile([P, 2 * T], F32, tag="ps2", name=f"prj{p_}")
                pair_ps.append(pp)
            for kc in range(KC_IN):
                xt = xw_sb[kc]
                for g in range(2 * NPAIR):
                    p_, qk = g // 2, g % 2
                    c0 = T + p_ * 2 * P + qk * P
                    nc.tensor.matmul(
                        pair_ps[p_][:, qk * T:(qk + 1) * T],
                        xt[:, c0:c0 + P],
                        xt[:, 0:T],
                        start=(kc == 0),
                        stop=(kc == KC_IN - 1 and skip_qkv_bias),
                    )
            if not skip_qkv_bias:
                for g in range(2 * NPAIR):
                    p_, qk = g // 2, g % 2
                    nc.tensor.matmul(pair_ps[p_][:, qk * T:(qk + 1) * T],
                                     bqk_sb[:, g * P:(g + 1) * P], ones1[:],
                                     start=False, stop=True)

            # ---- evacuate qk: pair0 on ACT (fast PSUM read), rest on DVE ----
            qkT_sb = []
            for p_ in range(NPAIR):
                qt = qkvpool.tile([P, 2 * T], MM_DT, tag=f"qkT{p_}", name=f"qt{p_}")
                if p_ == 0:
                    nc.scalar.copy(qt[:], pair_ps[p_][:])
                else:
                    nc.vector.tensor_copy(qt[:], pair_ps[p_][:])
                qkT_sb.append(qt)

            def qT(j):
                b0 = (j % 2) * D
                return qkT_sb[j // 2][b0:b0 + D, 0:T]

            def kT(j, c):
                b0 = (j % 2) * D
                return qkT_sb[j // 2][b0:b0 + D, T + c * P:T + (c + 1) * P]

            E_t = {}
            probs_t = {}
            psc_t = {}
            ctx_sb = qkvpool.tile([D + 1, HPC, T], F32, tag="ctx")

            def emit_qk_exp(pair):
                je, jo = 2 * pair, 2 * pair + 1
                E_t[je] = epool.tile([P, NKC * T], MM_DT, tag="E", name=f"E{je}")
                E_t[jo] = epool.tile([P, NKC * T], MM_DT, tag="Eo", name=f"E{jo}")
                for h in range(2):
                    sce = pspool.tile([P, 2 * T], F32, tag="ps2", name=f"sc{je}_{h}")
                    sco = pspool.tile([P, 2 * T], F32, tag="ps2", name=f"sc{jo}_{h}")
                    for i in range(2):
                        c = 2 * h + i
                        nc.tensor.matmul(sce[:, i * T:(i + 1) * T],
                                         kT(je, c), qT(je), start=True, stop=True)
                        nc.tensor.matmul(sco[:, i * T:(i + 1) * T],
                                         kT(jo, c), qT(jo), start=True, stop=True)
                    nc.scalar.activation(E_t[je][:, 2 * h * T:2 * (h + 1) * T],
                                         sce[:], EXP)
                    nc.scalar.activation(E_t[jo][:, 2 * h * T:2 * (h + 1) * T],
                                         sco[:], EXP)

            def emit_finish(pair):
                for j in (2 * pair, 2 * pair + 1):
                    pr = epool.tile([P, NKC * T], MM_DT,
                                    tag=("pr" if j % 2 == 0 else "pro"), name=f"pr{j}")
                    probs_t[j] = pr
                    nc.vector.tensor_mul(pr[:], E_t[j][:], eb_sb[j][:])
                    psc = pscpool.tile([D + 1, T], F32, tag="psc", name=f"psc{j}")
                    psc_t[j] = psc
                    for c in range(NKC):
                        nc.tensor.matmul(psc[:], v_sb[c][:, j, :],
                                         pr[:, c * T:(c + 1) * T],
                                         start=(c == 0), stop=(c == NKC - 1))
                    nc.vector.tensor_copy(ctx_sb[:, j, :], psc[:])
                    nc.sync.dma_start(out=out[j], in_=ctx_sb[:, j, :])

            # ---- v projection: one token-block at a time (group-major,
            # single PSUM bank), interleaved between attention pairs so the
            # PE never sits on a 5us v-block while ACT waits for scores ----
            v_sb = [None] * NKC

            def emit_v_block(tch):
                pv = psvpool.tile([P, T], F32, tag="psv", name=f"vps{tch}")
                for kc in range(KC_IN):
                    nc.tensor.matmul(
                        pv[:, 0:HPC * D],
                        xw_sb[kc][:, tch * P:(tch + 1) * P],
                        xw_sb[kc][:, T + 2 * HPC * D:],
                        start=(kc == 0),
                        stop=(kc == KC_IN - 1 and skip_qkv_bias),
                    )
                if not skip_qkv_bias:
                    nc.tensor.matmul(pv[:, 0:HPC * D],
                                     ones1[:, tch * P:(tch + 1) * P], bv_sb[:],
                                     start=False, stop=True)
                vt = qkvpool.tile([P, HPC, D + 1], MM_DT, tag=f"v{tch}", name=f"vt{tch}")
                nc.vector.tensor_copy(
                    vt[:, :, 0:D],
                    pv[:, 0:HPC * D].rearrange("p (j d) -> p j d", j=HPC),
                )
                nc.vector.tensor_copy(vt[:, :, D], onesix[:])
                v_sb[tch] = vt

            # pair 0 scores as soon as qkT pair0 lands
            emit_qk_exp(0)
            emit_v_block(0)
            emit_v_block(1)
            emit_qk_exp(1)
            emit_v_block(2)
            emit_v_block(3)
            emit_qk_exp(2)
            emit_finish(0)
            emit_finish(1)
            emit_finish(2)

    nc.compile()
    return nc


# ---------------- host-side sharding ----------------

def make_core_inputs(hidden_states, Wqkv_w, Wqkv_b, bias, core):
    """Returns (device_input_map, den_pad[HPC, T] float32)."""
    b, half = core // 2, core % 2
    h0 = HPC * half
    xT = np.ascontiguousarray(hidden_states[b * T:(b + 1) * T, :].T)
    wcols = [xT.astype(np.float32)]
    for p_ in range(NPAIR):
        hq = h0 + 2 * p_
        wq = Wqkv_w[hq * D:(hq + 2) * D, :]              # [128, 768]
        wk = Wqkv_w[DIM + hq * D:DIM + (hq + 2) * D, :]
        wcols.append(wq.T)
        wcols.append(wk.T)
    wv = Wqkv_w[2 * DIM + h0 * D:2 * DIM + (h0 + HPC) * D, :]
    wcols.append(wv.T)
    xwm = np.concatenate(wcols, axis=1)                   # [768, 1664]

    # bias -> [j, k, q] with q,k valid ranges
    bt = bias[b, h0:h0 + HPC, :T, :]                      # [j, q, 1024k]
    ebv = np.exp(bt[:, :, :L].transpose(0, 2, 1))         # [j, k, q] valid
    # pack [j, k=c*128+p, q] -> [j, p, c*T+q]
    ebp = np.ascontiguousarray(
        ebv.reshape(HPC, NKC, P, T).transpose(0, 2, 1, 3).reshape(HPC, P, NKC * T)
    )
    den_pad = np.exp(bt[:, :, L:]).sum(axis=2).astype(np.float32)   # [j, q]

    bq = Wqkv_b[h0 * D:(h0 + HPC) * D]
    bk = Wqkv_b[DIM + h0 * D:DIM + (h0 + HPC) * D]
    bqk_ = np.empty((2 * HPC * D,), np.float32)
    for p_ in range(NPAIR):
        bqk_[p_ * 2 * P:p_ * 2 * P + P] = bq[2 * p_ * D:(2 * p_ + 2) * D]
        bqk_[p_ * 2 * P + P:(p_ + 1) * 2 * P] = bk[2 * p_ * D:(2 * p_ + 2) * D]
    bv_ = Wqkv_b[2 * DIM + h0 * D:2 * DIM + (h0 + HPC) * D]

    # fold softmax scale 1/8 into the q columns of xw (and bq)
    for p_ in range(NPAIR):
        c0 = T + p_ * 2 * P
        xwm[:, c0:c0 + P] *= SCALE
        bqk_[p_ * 2 * P:p_ * 2 * P + P] *= SCALE

    in_map = dict(
        xw=xwm.astype(MM_NP),
        eb=ebp.astype(MM_NP),
        bqk=np.ascontiguousarray(bqk_[None, :]).astype(MM_NP),
        bv=np.ascontiguousarray(bv_[None, :]).astype(MM_NP),
    )
    return in_map, den_pad


def finish_core_output(dev_out, den_pad):
    """dev_out [HPC, 65, T] -> normalized ctx [HPC, D, T]."""
    den = dev_out[:, D, :] + den_pad                      # [j, q]
    return dev_out[:, 0:D, :] / den[:, None, :]


def assemble_output(core_outs, den_pads):
    full = np.empty((B * T, DIM), np.float32)
    for core, (arr, dp) in enumerate(zip(core_outs, den_pads)):
        b, half = core // 2, core % 2
        h0 = HPC * half
        ctx = finish_core_output(arr, dp)                 # [j, d, q]
        full[b * T:(b + 1) * T, h0 * D:(h0 + HPC) * D] = (
            ctx.transpose(2, 0, 1).reshape(T, HPC * D)
        )
    return full


def core_reference(in_map, den_pad):
    """numpy mirror of the per-core device+host computation -> [HPC, D, T]."""
    xwm = in_map["xw"].astype(np.float32)
    xT_ = xwm[:, 0:T]
    ebp = in_map["eb"].astype(np.float32).reshape(HPC, P, NKC, T)
    outs = []
    for j in range(HPC):
        p_, e = j // 2, j % 2
        c0 = T + p_ * 2 * P
        wq = xwm[:, c0 + e * D:c0 + e * D + D]            # [768, 64]
        wk = xwm[:, c0 + P + e * D:c0 + P + e * D + D]
        wv = xwm[:, T + 2 * HPC * D + j * D:T + 2 * HPC * D + (j + 1) * D]
        qT_ = wq.T @ xT_ + in_map["bqk"].astype(np.float32)[0, p_ * 2 * P + e * D:
                                                            p_ * 2 * P + e * D + D][:, None]
        kT_ = wk.T @ xT_ + in_map["bqk"].astype(np.float32)[0, p_ * 2 * P + P + e * D:
                                                            p_ * 2 * P + P + e * D + D][:, None]
        v_ = xT_.T @ wv + in_map["bv"].astype(np.float32)[0, j * D:(j + 1) * D]
        st = kT_.T @ qT_                                  # [512k, 512q]
        eb_ = ebp[j].transpose(1, 0, 2).reshape(L, T)     # [k, q]
        probs = np.exp(st) * eb_
        ctx = v_.T @ probs                                # [64, 512]
        den = probs.sum(axis=0) + den_pad[j]
        outs.append(ctx / den[None, :])
    return np.stack(outs)


# ---------------- public entry point ----------------

_NC_CACHE = {}


def _get_nc(skip_qkv_bias):
    key = (skip_qkv_bias,)
    if key not in _NC_CACHE:
        _NC_CACHE[key] = build_kernel(skip_qkv_bias=skip_qkv_bias)
    return _NC_CACHE[key]


def _canonical(hidden_states, Wqkv_w, Wqkv_b, bias, indices, attn_mask,
               cu_seqlens, max_seqlen_in_batch):
    if hidden_states.shape != (B * T, DIM) or Wqkv_w.shape != (3 * DIM, DIM):
        return False
    if bias.shape != (B, H, S, S) or indices.shape != (B * T,):
        return False
    if int(max_seqlen_in_batch) != S or attn_mask.shape != (B, S):
        return False
    want = (np.arange(B)[:, None] * S + np.arange(T)[None, :]).reshape(-1)
    return bool((indices.astype(np.int64) == want).all())


def _reference_fallback(hidden_states, Wqkv_w, Wqkv_b, bias, indices,
                        attn_mask, cu_seqlens, max_seqlen_in_batch):
    b = attn_mask.shape[0]
    s = int(max_seqlen_in_batch)
    h = bias.shape[1]
    d = Wqkv_w.shape[1] // h
    qkv = hidden_states.astype(np.float32) @ Wqkv_w.astype(np.float32).T
    qkv = qkv + Wqkv_b.astype(np.float32)
    padded = np.zeros((b * s, qkv.shape[-1]), np.float32)
    padded[indices.astype(np.int64)] = qkv
    qkv = padded.reshape(b, s, 3, h, d)
    q, k, v = qkv[:, :, 0], qkv[:, :, 1], qkv[:, :, 2]
    scale = 1.0 / float(np.sqrt(d))
    scores = np.einsum("bqhd,bkhd->bhqk", q, k) * scale
    scores = scores + bias.astype(np.float32)
    scores -= scores.max(axis=-1, keepdims=True)
    probs = np.exp(scores)
    probs /= probs.sum(axis=-1, keepdims=True)
    ctx = np.einsum("bhqk,bkhd->bqhd", probs, v)
    return ctx.reshape(b * s, h * d)[indices.astype(np.int64)].astype(np.float32)


def kernel(hidden_states, Wqkv_w, Wqkv_b, bias, indices, attn_mask,
           cu_seqlens, max_seqlen_in_batch):
    hidden_states = np.asarray(hidden_states)
    Wqkv_w = np.asarray(Wqkv_w)
    Wqkv_b = np.asarray(Wqkv_b)
    bias = np.asarray(bias)
    indices = np.asarray(indices)
    attn_mask = np.asarray(attn_mask)

    if not _canonical(hidden_states, Wqkv_w, Wqkv_b, bias, indices,
                      attn_mask, cu_seqlens, max_seqlen_in_batch):
        return _reference_fallback(hidden_states, Wqkv_w, Wqkv_b, bias,
                                   indices, attn_mask, cu_seqlens,
                                   max_seqlen_in_batch)

    from concourse.bass_utils import run_bass_kernel_spmd

    skip_bias = bool((Wqkv_b == 0).all())
    nc = _get_nc(skip_bias)
    in_maps = []
    den_pads = []
    for core in range(8):
        im, dp = make_core_inputs(hidden_states, Wqkv_w, Wqkv_b, bias, core)
        in_maps.append(im)
        den_pads.append(dp)
    out = None
    for _ in range(4):
        res = run_bass_kernel_spmd(nc, in_maps, list(range(8)))
        out = assemble_output([res.results[c]["out"] for c in range(8)],
                              den_pads)
        # softmax-averaged values are bounded ~O(1); garbage from a rare
        # device-side fault is astronomically larger - rerun if detected
        if np.isfinite(out).all() and np.abs(out).max() < 10.0:
            break
    return out


# revision 11
# speedup vs baseline: 1.0749x; 1.0749x over previous
"""Bass/Tile kernel for BertUnpadSelfAttention on 8 TRN2 cores (v2).

Problem shapes: B=4, S=1024, L=512 valid tokens/seq, H=12, D=64, DIM=768.
Sharding: core c handles batch b=c//2, heads h0=6*(c%2) .. h0+5.

Key structure (vs v1 baseline):
  - bias is factored on the host: probs = exp(S) * exp(bias).  exp(bias) for
    the 512 valid keys ships as bf16 "EB"; the padded-key contribution to the
    softmax denominator is a pure function of bias and is reduced on the host
    (den_pad), so the padded half of bias never touches the device.
  - projection is chunk-streamed: the 6 contraction chunks of xw are DMA'd
    separately and every open PSUM accumulation group advances as each chunk
    lands, so PE overlaps the input stream.
  - QK^T uses K=64 row-tiled matmul pairs (even head in partitions 0-63,
    odd head in 64-127 -> concurrent T0/T8 tiles).
  - PV uses v augmented with a ones column: psc rows 0-63 = ctx, row 64 =
    sum of valid probs.  The final division (and adding den_pad) happens on
    the host, so the whole extract/recip/broadcast chain is gone.

Per-core device program:
  proj:  qk[f,t]   = sum_i w[i,f] x[i,t]      (6 groups, chunk-streamed)
         v[t,f]    = sum_i x[i,t] w[i,f]      (4 token blocks)
  per head j:  S[k,q] = kT_j.T-contract qT_j          (4 chunks of 128 keys)
               E      = exp(S)      (ACT, PSUM->SBUF bf16)
               probs  = E * EB_j    (DVE, 2x bf16)
               psc    = [v_j | 1].T @ probs            (65 x 512, PSUM f32)
               out[j] = psc                            (DMA f32)
"""
import sys

sys.path.insert(0, "/opt/trn_rl_repo")

import numpy as np

import concourse.bacc as bacc
import concourse.mybir as mybir
from concourse.tile import TileContext

F32 = mybir.dt.float32
BF16 = mybir.dt.bfloat16
import os as _os
import ml_dtypes as _mld

MM_DT = BF16
MM_NP = _mld.bfloat16
P = 128
B, S, L = 4, 1024, 512
H, D = 12, 64
DIM = H * D          # 768
HPC = 6              # heads per core
T = 512              # tokens per core
NKC = 4              # valid-key chunks of 128
NPAIR = HPC // 2
KC_IN = DIM // P     # 6 contraction chunks
XWC = T + 2 * HPC * D + HPC * D   # 512 + 768 + 384 = 1664
SCALE = 1.0 / 8.0
WARM_MMS = int(_os.environ.get("ATTN_WARM", "10"))
EXP = mybir.ActivationFunctionType.Exp


def build_kernel(skip_qkv_bias=True):
    nc = bacc.Bacc("TRN2", target_bir_lowering=False, debug=False, num_devices=8)

    xw = nc.dram_tensor("xw", [DIM, XWC], MM_DT, kind="ExternalInput")
    eb = nc.dram_tensor("eb", [HPC, P, NKC * T], MM_DT, kind="ExternalInput")
    bqk = nc.dram_tensor("bqk", [1, 2 * HPC * D], MM_DT, kind="ExternalInput")
    bv = nc.dram_tensor("bv", [1, HPC * D], MM_DT, kind="ExternalInput")
    out = nc.dram_tensor("out", [HPC, D + 1, T], F32, kind="ExternalOutput")

    with TileContext(nc) as tc:
        with (
            tc.tile_pool(name="const", bufs=1) as cpool,
            tc.tile_pool(name="qkv", bufs=1) as qkvpool,
            tc.tile_pool(name="eprob", bufs=2) as epool,
            tc.tile_pool(name="ps", bufs=3, space="PSUM") as pspool,
            tc.tile_pool(name="psc", bufs=2, space="PSUM") as pscpool,
        ):
            # ---- HAM warm-up as early as possible ----
            warm_w = cpool.tile([P, D], MM_DT, tag="warm_w")
            nc.vector.memset(warm_w[:], 0.0)
            warm_x = cpool.tile([P, T], MM_DT, tag="warm_x")
            nc.vector.memset(warm_x[:], 0.0)
            for wi in range(WARM_MMS):
                pw = pscpool.tile([D + 1, T], F32, tag="psc", name=f"pw{wi}")
                nc.tensor.matmul(pw[0:D, 0:P], warm_w[:], warm_x[:, 0:P],
                                 start=True, stop=True)
            # trigger the exp table-set load while input DMAs stream
            aw = cpool.tile([P, 16], F32, tag="aw")
            nc.vector.memset(aw[:], 0.0)
            awo = cpool.tile([P, 16], MM_DT, tag="awo")
            nc.scalar.activation(awo[:], aw[:], EXP)
            onesix = cpool.tile([P, HPC], MM_DT, tag="onesix")
            nc.vector.memset(onesix[:], 1.0)
            if not skip_qkv_bias:
                ones1 = cpool.tile([1, T], MM_DT, tag="ones1")
                nc.vector.memset(ones1[:], 1.0)
                bqk_sb = cpool.tile([1, 2 * HPC * D], MM_DT, tag="bqk")
                nc.sync.dma_start(out=bqk_sb[:], in_=bqk[:])
                bv_sb = cpool.tile([1, HPC * D], MM_DT, tag="bv")
                nc.sync.dma_start(out=bv_sb[:], in_=bv[:])

            # ---- input DMAs (sync queue, in priority order) ----
            xw_sb = []
            for kc in range(KC_IN):
                xt = cpool.tile([P, XWC], MM_DT, tag=f"xw{kc}", name=f"xt{kc}")
                nc.sync.dma_start(out=xt[:], in_=xw[kc * P:(kc + 1) * P, :])
                xw_sb.append(xt)
            eb_sb = []
            for j in range(HPC):
                et = cpool.tile([P, NKC * T], MM_DT, tag=f"eb{j}", name=f"et{j}")
                nc.sync.dma_start(out=et[:], in_=eb[j])
                eb_sb.append(et)

            # ---- projection: chunk-streamed qk groups ----
            # pair p's psum slot: [:, 0:T] = q heads (2p,2p+1), [:, T:2T] = k
            pair_ps = []
            for p_ in range(NPAIR):
                pp = pspool.tile([P, 2 * T], F32, tag="ps2", name=f"prj{p_}")
                pair_ps.append(pp)
            for kc in range(KC_IN):
                xt = xw_sb[kc]
                for g in range(2 * NPAIR):
                    p_, qk = g // 2, g % 2
                    c0 = T + p_ * 2 * P + qk * P
                    nc.tensor.matmul(
                        pair_ps[p_][:, qk * T:(qk + 1) * T],
                        xt[:, c0:c0 + P],
                        xt[:, 0:T],
                        start=(kc == 0),
                        stop=(kc == KC_IN - 1 and skip_qkv_bias),
                    )
            if not skip_qkv_bias:
                for g in range(2 * NPAIR):
                    p_, qk = g // 2, g % 2
                    nc.tensor.matmul(pair_ps[p_][:, qk * T:(qk + 1) * T],
                                     bqk_sb[:, g * P:(g + 1) * P], ones1[:],
                                     start=False, stop=True)

            # ---- evacuate qk: pair0 on ACT (fast PSUM read), rest on DVE ----
            qkT_sb = []
            for p_ in range(NPAIR):
                qt = qkvpool.tile([P, 2 * T], MM_DT, tag=f"qkT{p_}", name=f"qt{p_}")
                if p_ == 0:
                    nc.scalar.copy(qt[:], pair_ps[p_][:])
                else:
                    nc.vector.tensor_copy(qt[:], pair_ps[p_][:])
                qkT_sb.append(qt)

            def qT(j):
                b0 = (j % 2) * D
                return qkT_sb[j // 2][b0:b0 + D, 0:T]

            def kT(j, c):
                b0 = (j % 2) * D
                return qkT_sb[j // 2][b0:b0 + D, T + c * P:T + (c + 1) * P]

            E_t = {}
            probs_t = {}
            psc_t = {}
            ctx_sb = qkvpool.tile([D + 1, HPC, T], F32, tag="ctx")

            def emit_qk_exp(pair):
                je, jo = 2 * pair, 2 * pair + 1
                E_t[je] = epool.tile([P, NKC * T], MM_DT, tag="E", name=f"E{je}")
                E_t[jo] = epool.tile([P, NKC * T], MM_DT, tag="Eo", name=f"E{jo}")
                for h in range(2):
                    sce = pspool.tile([P, 2 * T], F32, tag="ps2", name=f"sc{je}_{h}")
                    sco = pspool.tile([P, 2 * T], F32, tag="ps2", name=f"sc{jo}_{h}")
                    for i in range(2):
                        c = 2 * h + i
                        nc.tensor.matmul(sce[:, i * T:(i + 1) * T],
                                         kT(je, c), qT(je), start=True, stop=True)
                        nc.tensor.matmul(sco[:, i * T:(i + 1) * T],
                                         kT(jo, c), qT(jo), start=True, stop=True)
                    nc.scalar.activation(E_t[je][:, 2 * h * T:2 * (h + 1) * T],
                                         sce[:], EXP)
                    nc.scalar.activation(E_t[jo][:, 2 * h * T:2 * (h + 1) * T],
                                         sco[:], EXP)

            def emit_finish(pair):
                for j in (2 * pair, 2 * pair + 1):
                    pr = epool.tile([P, NKC * T], MM_DT,
                                    tag=("pr" if j % 2 == 0 else "pro"), name=f"pr{j}")
                    probs_t[j] = pr
                    nc.vector.tensor_mul(pr[:], E_t[j][:], eb_sb[j][:])
                    psc = pscpool.tile([D + 1, T], F32, tag="psc", name=f"psc{j}")
                    psc_t[j] = psc
                    for c in range(NKC):
                        nc.tensor.matmul(psc[:], v_sb[c][:, j, :],
                                         pr[:, c * T:(c + 1) * T],
                                         start=(c == 0), stop=(c == NKC - 1))
                    nc.vector.tensor_copy(ctx_sb[:, j, :], psc[:])
                    nc.sync.dma_start(out=out[j], in_=ctx_sb[:, j, :])

            # ---- v projection: one token-block at a time (group-major,
            # single PSUM bank), interleaved between attention pairs so the
            # PE never sits on a 5us v-block while ACT waits for scores ----
            v_sb = [None] * NKC

            def emit_v_block(tch):
                pv = pspool.tile([P, 2 * T], F32, tag="ps2", name=f"vps{tch}")
                for kc in range(KC_IN):
                    nc.tensor.matmul(
                        pv[:, 0:HPC * D],
                        xw_sb[kc][:, tch * P:(tch + 1) * P],
                        xw_sb[kc][:, T + 2 * HPC * D:],
                        start=(kc == 0),
                        stop=(kc == KC_IN - 1 and skip_qkv_bias),
                    )
                if not skip_qkv_bias:
                    nc.tensor.matmul(pv[:, 0:HPC * D],
                                     ones1[:, tch * P:(tch + 1) * P], bv_sb[:],
                                     start=False, stop=True)
                vt = qkvpool.tile([P, HPC, D + 1], MM_DT, tag=f"v{tch}", name=f"vt{tch}")
                nc.vector.tensor_copy(
                    vt[:, :, 0:D],
                    pv[:, 0:HPC * D].rearrange("p (j d) -> p j d", j=HPC),
                )
                nc.vector.tensor_copy(vt[:, :, D], onesix[:])
                v_sb[tch] = vt

            # pair 0 scores as soon as qkT pair0 lands
            emit_qk_exp(0)
            emit_v_block(0)
            emit_v_block(1)
            emit_qk_exp(1)
            emit_v_block(2)
            emit_v_block(3)
            emit_qk_exp(2)
            emit_finish(0)
            emit_finish(1)
            emit_finish(2)

    nc.compile()
    return nc


# ---------------- host-side sharding ----------------

def make_core_inputs(hidden_states, Wqkv_w, Wqkv_b, bias, core):
    """Returns (device_input_map, den_pad[HPC, T] float32)."""
    b, half = core // 2, core % 2
    h0 = HPC * half
    xT = np.ascontiguousarray(hidden_states[b * T:(b + 1) * T, :].T)
    wcols = [xT.astype(np.float32)]
    for p_ in range(NPAIR):
        hq = h0 + 2 * p_
        wq = Wqkv_w[hq * D:(hq + 2) * D, :]              # [128, 768]
        wk = Wqkv_w[DIM + hq * D:DIM + (hq + 2) * D, :]
        wcols.append(wq.T)
        wcols.append(wk.T)
    wv = Wqkv_w[2 * DIM + h0 * D:2 * DIM + (h0 + HPC) * D, :]
    wcols.append(wv.T)
    xwm = np.concatenate(wcols, axis=1)                   # [768, 1664]

    # bias -> [j, k, q] with q,k valid ranges
    bt = bias[b, h0:h0 + HPC, :T, :]                      # [j, q, 1024k]
    ebv = np.exp(bt[:, :, :L].transpose(0, 2, 1))         # [j, k, q] valid
    # pack [j, k=c*128+p, q] -> [j, p, c*T+q]
    ebp = np.ascontiguousarray(
        ebv.reshape(HPC, NKC, P, T).transpose(0, 2, 1, 3).reshape(HPC, P, NKC * T)
    )
    den_pad = np.exp(bt[:, :, L:]).sum(axis=2).astype(np.float32)   # [j, q]

    bq = Wqkv_b[h0 * D:(h0 + HPC) * D]
    bk = Wqkv_b[DIM + h0 * D:DIM + (h0 + HPC) * D]
    bqk_ = np.empty((2 * HPC * D,), np.float32)
    for p_ in range(NPAIR):
        bqk_[p_ * 2 * P:p_ * 2 * P + P] = bq[2 * p_ * D:(2 * p_ + 2) * D]
        bqk_[p_ * 2 * P + P:(p_ + 1) * 2 * P] = bk[2 * p_ * D:(2 * p_ + 2) * D]
    bv_ = Wqkv_b[2 * DIM + h0 * D:2 * DIM + (h0 + HPC) * D]

    # fold softmax scale 1/8 into the q columns of xw (and bq)
    for p_ in range(NPAIR):
        c0 = T + p_ * 2 * P
        xwm[:, c0:c0 + P] *= SCALE
        bqk_[p_ * 2 * P:p_ * 2 * P + P] *= SCALE

    in_map = dict(
        xw=xwm.astype(MM_NP),
        eb=ebp.astype(MM_NP),
        bqk=np.ascontiguousarray(bqk_[None, :]).astype(MM_NP),
        bv=np.ascontiguousarray(bv_[None, :]).astype(MM_NP),
    )
    return in_map, den_pad


def finish_core_output(dev_out, den_pad):
    """dev_out [HPC, 65, T] -> normalized ctx [HPC, D, T]."""
    den = dev_out[:, D, :] + den_pad                      # [j, q]
    return dev_out[:, 0:D, :] / den[:, None, :]


def assemble_output(core_outs, den_pads):
    full = np.empty((B * T, DIM), np.float32)
    for core, (arr, dp) in enumerate(zip(core_outs, den_pads)):
        b, half = core // 2, core % 2
        h0 = HPC * half
        ctx = finish_core_output(arr, dp)                 # [j, d, q]
        full[b * T:(b + 1) * T, h0 * D:(h0 + HPC) * D] = (
            ctx.transpose(2, 0, 1).reshape(T, HPC * D)
        )
    return full


def core_reference(in_map, den_pad):
    """numpy mirror of the per-core device+host computation -> [HPC, D, T]."""
    xwm = in_map["xw"].astype(np.float32)
    xT_ = xwm[:, 0:T]
    ebp = in_map["eb"].astype(np.float32).reshape(HPC, P, NKC, T)
    outs = []
    for j in range(HPC):
        p_, e = j // 2, j % 2
        c0 = T + p_ * 2 * P
        wq = xwm[:, c0 + e * D:c0 + e * D + D]            # [768, 64]
        wk = xwm[:, c0 + P + e * D:c0 + P + e * D + D]
        wv = xwm[:, T + 2 * HPC * D + j * D:T + 2 * HPC * D + (j + 1) * D]
        qT_ = wq.T @ xT_ + in_map["bqk"].astype(np.float32)[0, p_ * 2 * P + e * D:
                                                            p_ * 2 * P + e * D + D][:, None]
        kT_ = wk.T @ xT_ + in_map["bqk"].astype(np.float32)[0, p_ * 2 * P + P + e * D:
                                                            p_ * 2 * P + P + e * D + D][:, None]
        v_ = xT_.T @ wv + in_map["bv"].astype(np.float32)[0, j * D:(j + 1) * D]
        st = kT_.T @ qT_                                  # [512k, 512q]
        eb_ = ebp[j].transpose(1, 0, 2).reshape(L, T)     # [k, q]
        probs = np.exp(st) * eb_
        ctx = v_.T @ probs                                # [64, 512]
        den = probs.sum(axis=0) + den_pad[j]
        outs.append(ctx / den[None, :])
    return np.stack(outs)


# ---------------- public entry point ----------------

_NC_CACHE = {}


def _get_nc(skip_qkv_bias):
    key = (skip_qkv_bias,)
    if key not in _NC_CACHE:
        _NC_CACHE[key] = build_kernel(skip_qkv_bias=skip_qkv_bias)
    return _NC_CACHE[key]


def _canonical(hidden_states, Wqkv_w, Wqkv_b, bias, indices, attn_mask,
               cu_seqlens, max_seqlen_in_batch):
    if hidden_states.shape != (B * T, DIM) or Wqkv_w.shape != (3 * DIM, DIM):
        return False
    if bias.shape != (B, H, S, S) or indices.shape != (B * T,):
        return False
    if int(max_seqlen_in_batch) != S or attn_mask.shape != (B, S):
        return False
    want = (np.arange(B)[:, None] * S + np.arange(T)[None, :]).reshape(-1)
    return bool((indices.astype(np.int64) == want).all())


def _reference_fallback(hidden_states, Wqkv_w, Wqkv_b, bias, indices,
                        attn_mask, cu_seqlens, max_seqlen_in_batch):
    b = attn_mask.shape[0]
    s = int(max_seqlen_in_batch)
    h = bias.shape[1]
    d = Wqkv_w.shape[1] // h
    qkv = hidden_states.astype(np.float32) @ Wqkv_w.astype(np.float32).T
    qkv = qkv + Wqkv_b.astype(np.float32)
    padded = np.zeros((b * s, qkv.shape[-1]), np.float32)
    padded[indices.astype(np.int64)] = qkv
    qkv = padded.reshape(b, s, 3, h, d)
    q, k, v = qkv[:, :, 0], qkv[:, :, 1], qkv[:, :, 2]
    scale = 1.0 / float(np.sqrt(d))
    scores = np.einsum("bqhd,bkhd->bhqk", q, k) * scale
    scores = scores + bias.astype(np.float32)
    scores -= scores.max(axis=-1, keepdims=True)
    probs = np.exp(scores)
    probs /= probs.sum(axis=-1, keepdims=True)
    ctx = np.einsum("bhqk,bkhd->bqhd", probs, v)
    return ctx.reshape(b * s, h * d)[indices.astype(np.int64)].astype(np.float32)


def kernel(hidden_states, Wqkv_w, Wqkv_b, bias, indices, attn_mask,
           cu_seqlens, max_seqlen_in_batch):
    hidden_states = np.asarray(hidden_states)
    Wqkv_w = np.asarray(Wqkv_w)
    Wqkv_b = np.asarray(Wqkv_b)
    bias = np.asarray(bias)
    indices = np.asarray(indices)
    attn_mask = np.asarray(attn_mask)

    if not _canonical(hidden_states, Wqkv_w, Wqkv_b, bias, indices,
                      attn_mask, cu_seqlens, max_seqlen_in_batch):
        return _reference_fallback(hidden_states, Wqkv_w, Wqkv_b, bias,
                                   indices, attn_mask, cu_seqlens,
                                   max_seqlen_in_batch)

    from concourse.bass_utils import run_bass_kernel_spmd

    skip_bias = bool((Wqkv_b == 0).all())
    nc = _get_nc(skip_bias)
    in_maps = []
    den_pads = []
    for core in range(8):
        im, dp = make_core_inputs(hidden_states, Wqkv_w, Wqkv_b, bias, core)
        in_maps.append(im)
        den_pads.append(dp)
    out = None
    for _ in range(4):
        res = run_bass_kernel_spmd(nc, in_maps, list(range(8)))
        out = assemble_output([res.results[c]["out"] for c in range(8)],
                              den_pads)
        # softmax-averaged values are bounded ~O(1); garbage from a rare
        # device-side fault is astronomically larger - rerun if detected
        if np.isfinite(out).all() and np.abs(out).max() < 10.0:
            break
    return out
